# revision 2
# baseline (speedup 1.0000x reference)
"""UNet kernel for 8 Trainium2 NeuronCores.

Sharding: data-parallel over batch (B=8 -> 1 element per core) via a
single SPMD pmap program. All conv/pool/upsample stages are
batch-independent, so no collectives are needed.

FCAS: the rank op touches only batch element 0, channel 1. Its output
is (p*w0+b0 + n*w1+b1 + e*w2+b2)/3 with p+n+e == N always. When
w0==w1==w2 (the shipped weights) the value is the data-independent
constant (w0*N + b0+b1+b2)/3, so FCAS reduces to an elementwise blend
x4*mult+add with host-built maps (identity maps on cores 1-7). For
general unequal weights a two-stage path runs the encoder on device,
does the exact rank computation on host, and resumes on device.

Ops used on device: pad, slice, dot_general, elementwise, concat --
chosen for maximal neuronx-cc compatibility (conv is expressed as 9
shifted channel-contraction einsums; bilinear up2 as constant
interpolation matmuls).
"""
import numpy as np
import jax
import jax.numpy as jnp
from functools import partial

EPS = 1e-5
_BN = np.float32(1.0 / np.sqrt(1.0 + EPS))


def _conv3(x, w, b):
    # 3x3 SAME conv as 9 shifted channel-contraction matmuls.
    H, W = x.shape[2], x.shape[3]
    xp = jnp.pad(x, ((0, 0), (0, 0), (1, 1), (1, 1)))
    out = b[None, :, None, None] * jnp.ones_like(x[:, :1, :, :])  # broadcast later
    acc = None
    for dy in range(3):
        for dx in range(3):
            xs = xp[:, :, dy:dy + H, dx:dx + W]
            t = jnp.einsum('oi,nihw->nohw', w[:, :, dy, dx], xs)
            acc = t if acc is None else acc + t
    return acc + b[None, :, None, None]


def _conv1(x, w, b):
    return jnp.einsum('oi,nihw->nohw', w[:, :, 0, 0], x) + b[None, :, None, None]


def _cbr(x, w, b, g, a):
    y = _conv3(x, w, b)
    y = g[None, :, None, None] * (y * _BN) + a[None, :, None, None]
    return jnp.maximum(y, 0.0)


def _pool(x):
    a = x[:, :, 0::2, 0::2]
    b = x[:, :, 0::2, 1::2]
    c = x[:, :, 1::2, 0::2]
    d = x[:, :, 1::2, 1::2]
    return jnp.maximum(jnp.maximum(a, b), jnp.maximum(c, d))


def _up_mat(H):
    # align_corners=True bilinear 2x upsample as a dense [2H, H] matrix.
    Ho = 2 * H
    ys = np.arange(Ho) * ((H - 1) / (Ho - 1))
    y0 = np.floor(ys).astype(np.int64)
    y1 = np.minimum(y0 + 1, H - 1)
    wy = (ys - y0).astype(np.float32)
    U = np.zeros((Ho, H), np.float32)
    U[np.arange(Ho), y0] += (1.0 - wy)
    U[np.arange(Ho), y1] += wy
    return U


_U = {H: _up_mat(H) for H in (64, 128, 256)}


def _up2(x):
    H = x.shape[2]
    U = _U[H]
    t = jnp.einsum('oh,nihw->niow', U, x)
    return jnp.einsum('pw,niow->niop', U, t)


def _encoder(x, p):
    x1 = _cbr(x, p['w_inc'], p['b_inc'], p['g_inc'], p['a_inc'])
    x2 = _cbr(_pool(x1), p['w_d1'], p['b_d1'], p['g_d1'], p['a_d1'])
    x3 = _cbr(_pool(x2), p['w_d2'], p['b_d2'], p['g_d2'], p['a_d2'])
    x4 = _cbr(_pool(x3), p['w_d3'], p['b_d3'], p['g_d3'], p['a_d3'])
    return x1, x2, x3, x4


def _decoder(x1, x2, x3, x4, p):
    u = _cbr(jnp.concatenate([x3, _up2(x4)], axis=1), p['w_u2'], p['b_u2'], p['g_u2'], p['a_u2'])
    u = _cbr(jnp.concatenate([x2, _up2(u)], axis=1), p['w_u3'], p['b_u3'], p['g_u3'], p['a_u3'])
    u = _cbr(jnp.concatenate([x1, _up2(u)], axis=1), p['w_u4'], p['b_u4'], p['g_u4'], p['a_u4'])
    s = _conv1(u, p['w_out'], p['b_out'])
    return 1.0 / (1.0 + jnp.exp(-s))


def _forward_blend(x, mult, add, **p):
    # x: [1,3,512,512] local shard; mult/add: [32,64,64] FCAS blend maps.
    x1, x2, x3, x4 = _encoder(x, p)
    x4 = x4 * mult[None] + add[None]
    return _decoder(x1, x2, x3, x4, p)


def _enc_only(x, **p):
    return _encoder(x, p)


def _dec_only(x1, x2, x3, x4, **p):
    return _decoder(x1, x2, x3, x4, p)


_pm_forward = None
_pm_enc = None
_pm_dec = None


def _get_forward():
    global _pm_forward
    if _pm_forward is None:
        _pm_forward = jax.pmap(_forward_blend, in_axes=(0, 0, 0),
                               static_broadcasted_argnums=())
    return _pm_forward


def _host_fcas(x4_0, w, b):
    # exact numpy replica of the reference rank op on x4[0? -> given element]
    ch = x4_0[1]
    H, W = ch.shape
    flat = ch.ravel(); N = flat.size
    s = np.sort(flat)
    left = np.searchsorted(s, flat, side='left')
    right = np.searchsorted(s, flat, side='right')
    e = left.astype(np.float32)
    n = (right - left).astype(np.float32)
    p = (N - right).astype(np.float32)
    val = ((p * w[0] + b[0] + n * w[1] + b[1] + e * w[2] + b[2]) / 3.0).reshape(H, W)
    new_ch = ch.copy()
    new_ch[1:H - 1, 1:W - 1] = val[1:H - 1, 1:W - 1]
    out = x4_0.copy()
    out[1] = new_ch
    return out


def kernel(**inputs):
    x = np.asarray(inputs['x'], np.float32)
    B = x.shape[0]
    p = {k: np.asarray(v, np.float32) for k, v in inputs.items()
         if k not in ('x', 'fcas_w', 'fcas_b')}
    fw = np.asarray(inputs['fcas_w'], np.float32)
    fb = np.asarray(inputs['fcas_b'], np.float32)

    xs = x.reshape(B, 1, *x.shape[1:])
    # replicate params across devices via broadcast in_axes=None is not
    # supported uniformly; tile them on axis 0 instead (they are tiny).
    pp = {k: np.broadcast_to(v, (B,) + v.shape) for k, v in p.items()}

    if fw[0] == fw[1] == fw[2]:
        # FCAS value is constant: (w*N + sum(b))/3 on interior of ch 1.
        C = np.float32((fw[0] * 4096.0 + fb.sum()) / 3.0)
        mult = np.ones((B, 32, 64, 64), np.float32)
        add = np.zeros((B, 32, 64, 64), np.float32)
        mult[0, 1, 1:63, 1:63] = 0.0
        add[0, 1, 1:63, 1:63] = C
        out = _get_forward()(xs, mult, add, **pp)
        return np.asarray(out).reshape(B, 1, 512, 512).astype(np.float32)

    # general (unequal weights): exact two-stage path
    global _pm_enc, _pm_dec
    if _pm_enc is None:
        _pm_enc = jax.pmap(_enc_only)
        _pm_dec = jax.pmap(_dec_only)
    x1, x2, x3, x4 = _pm_enc(xs, **pp)
    x4 = np.asarray(x4)
    x4[0, 0] = _host_fcas(x4[0, 0], fw, fb)
    out = _pm_dec(x1, x2, x3, jnp.asarray(x4), **pp)
    return np.asarray(out).reshape(B, 1, 512, 512).astype(np.float32)
